# revision 12
# baseline (speedup 1.0000x reference)
"""EMA (first-order linear recurrence along T) for x[16, 512, 4096] f32.

y[..., 0] = x[..., 0];  y[..., t] = s_c * x[..., t] + (1 - s_c) * y[..., t-1]

Sharding: data-parallel over batch B across 8 cores (2 batches/core, each a
contiguous 16 MiB slab). Per core the (b, c) pairs form 1024 independent rows
of length T=4096; the recurrence maps 1:1 onto the TensorTensorScanArith
instruction (state = data0*state + data1 along the free dim, one recurrence
per partition).

Pipeline per 128-row block, all in-place on one SBUF tile X:
  DMA in -> ACT: X[:,1:] *= s (per-partition scale, scalar engine)
         -> scan: X = a*state + X with initial=0 (col 0 still holds raw x_0,
            so state_0 = x_0 exactly) -> DMA out.
Scans alternate Vector/GpSimd so neither engine reaches the DMA roofline
(~94 us for 33.5 MB per core); a lone Vector doing all 8 scans would sit at
~93 us busy and fight the DMA for the critical path.
"""

import numpy as np

import concourse.bacc as bacc
import concourse.mybir as mybir
import concourse.tile as tile
from concourse.bass_utils import run_bass_kernel_spmd

B, C, T = 16, 512, 4096
N_CORES = 8
B_PER = B // N_CORES          # 2 batches per core
ROWS = B_PER * C              # 1024 (b, c) rows per core
P = 128                       # SBUF partitions
N_BLOCKS = ROWS // P          # 8 row blocks per core
C_BLOCKS = C // P             # 4 channel blocks (weights layout)

DT = mybir.dt.float32
OP = mybir.AluOpType

BUFS = 4                      # 4 paired tiles of 32 KiB/partition, all in flight


def build(b_per=B_PER, c=C, t=T):
    rows = b_per * c
    n_blocks = rows // P
    c_blocks = c // P

    nc = bacc.Bacc("TRN2", target_bir_lowering=False, debug=False)

    x_in = nc.dram_tensor("x", [b_per, c, t], DT, kind="ExternalInput")
    w_in = nc.dram_tensor("weights", [c], DT, kind="ExternalInput")
    y_out = nc.dram_tensor("out", [b_per, c, t], DT, kind="ExternalOutput")

    xr = x_in.ap().rearrange("b c t -> (b c) t")   # [rows, t]
    yr = y_out.ap().rearrange("b c t -> (b c) t")
    # w4[p, j] = weights[j*128 + p] — column j holds channel block j
    wr = w_in.ap().rearrange("(j p) -> p j", p=P)  # [128, c_blocks]

    with tile.TileContext(nc) as tc:
        with (
            tc.tile_pool(name="const", bufs=1) as cpool,
            tc.tile_pool(name="xp", bufs=BUFS) as xpool,
        ):
            w4 = cpool.tile([P, c_blocks], DT)
            s4 = cpool.tile([P, c_blocks], DT)
            a4 = cpool.tile([P, c_blocks], DT)
            nc.sync.dma_start(w4[:], wr)
            # s = clamp(w, 0, 1); a = 1 - s  (gpsimd: keeps Vector scan-only)
            nc.gpsimd.tensor_scalar(s4[:], w4[:], 0.0, 1.0, OP.max, OP.min)
            nc.gpsimd.tensor_scalar(a4[:], s4[:], -1.0, 1.0, OP.mult, OP.add)

            # One tile per channel block j: holds rows (b=0, cblk j) in
            # cols [0, t) and (b=1, cblk j) in cols [t, 2t) — a single
            # 2*t-wide DMA each way (the b stride is affine in DRAM).
            # Fewer, larger DMAs sustain higher HBM BW.
            for j in range(c_blocks):
                xsrc = x_in.ap()[:, j * P:(j + 1) * P, :].transpose([1, 0, 2])
                ysrc = y_out.ap()[:, j * P:(j + 1) * P, :].transpose([1, 0, 2])
                xt = xpool.tile([P, b_per * t], DT)
                xt3 = xt[:].rearrange("c (b t) -> c b t", b=b_per)
                nc.sync.dma_start(xt3, xsrc)

                for h in range(b_per):
                    # Premultiply s*x in place, skipping col 0 of each half:
                    # the scan's first step then computes
                    # state_0 = a*0 + x_0 = x_0 exactly.
                    nc.scalar.activation(
                        xt[:, h * t + 1:(h + 1) * t], xt[:, h * t + 1:(h + 1) * t],
                        mybir.ActivationFunctionType.Copy,
                        scale=s4[:, j:j + 1],
                    )
                    nc.vector.tensor_tensor_scan(
                        xt[:, h * t:(h + 1) * t],
                        a4[:, j:j + 1].to_broadcast((P, t)),
                        xt[:, h * t:(h + 1) * t],
                        0.0,
                        OP.mult,
                        OP.add,
                    )
                # Out-DMAs issue from gpsimd: its own issue queue, so an
                # out (blocked on the scans) never head-of-line-blocks the
                # remaining in-DMAs on sync.
                nc.gpsimd.dma_start(ysrc, xt3)
    nc.compile()
    return nc


_NC_CACHE = []


def kernel(x, weights, _run_kwargs=None):
    if not _NC_CACHE:
        _NC_CACHE.append(build())
    nc = _NC_CACHE[0]
    x = np.ascontiguousarray(np.asarray(x, dtype=np.float32))
    weights = np.ascontiguousarray(np.asarray(weights, dtype=np.float32))
    in_maps = [
        {"x": x[i * B_PER:(i + 1) * B_PER], "weights": weights}
        for i in range(N_CORES)
    ]
    res = run_bass_kernel_spmd(
        nc, in_maps, core_ids=list(range(N_CORES)), **(_run_kwargs or {})
    )
    out = np.concatenate([res.results[i]["out"] for i in range(N_CORES)], axis=0)
    if _run_kwargs:
        kernel.last_results = res
    return out


# revision 14
# speedup vs baseline: 1.0314x; 1.0314x over previous
"""EMA (first-order linear recurrence along T) for x[16, 512, 4096] f32.

y[..., 0] = x[..., 0];  y[..., t] = s_c * x[..., t] + (1 - s_c) * y[..., t-1]

Sharding: data-parallel over batch B across 8 cores (2 batches/core, each a
contiguous 16 MiB slab). Per core the (b, c) pairs form 1024 independent rows
of length T=4096; the recurrence maps 1:1 onto the TensorTensorScanArith
instruction (state = data0*state + data1 along the free dim, one recurrence
per partition).

Per 128-row block, all in-place on one SBUF tile X:
  DMA in (sync queue) -> ACT: X[:,1:] *= s (per-partition scale, scalar
  engine) -> scan on Vector: X = a*state + X with initial=0 (col 0 still
  holds raw x_0, so state_0 = x_0 exactly) -> DMA out (gpsimd queue, so a
  blocked out never head-of-line-blocks the remaining in-DMAs on sync).

The kernel is wire-bound (~90 us for 33.5 MB/core at the measured
~373 GB/s) and the Vector scan chain (8 x 8.66 us) is co-critical, so the
first and last blocks are split into half-T pieces to shorten pipeline fill
and drain, the weights DMA issues from the Vector queue to keep the sync
ring free for x, and a dummy activation hoists the one-time ACT_TABLE_LOAD
into the engine preamble.
"""

import numpy as np

import concourse.bacc as bacc
import concourse.mybir as mybir
import concourse.tile as tile
from concourse.bass_utils import run_bass_kernel_spmd

B, C, T = 16, 512, 4096
N_CORES = 8
B_PER = B // N_CORES          # 2 batches per core
ROWS = B_PER * C              # 1024 (b, c) rows per core
P = 128                       # SBUF partitions
N_BLOCKS = ROWS // P          # 8 row blocks per core
C_BLOCKS = C // P             # 4 channel blocks (weights layout)

DT = mybir.dt.float32
OP = mybir.AluOpType
ACT_COPY = mybir.ActivationFunctionType.Copy


def build(b_per=B_PER, c=C, t=T):
    rows = b_per * c
    n_blocks = rows // P
    c_blocks = c // P
    th = t // 2

    nc = bacc.Bacc("TRN2", target_bir_lowering=False, debug=False)

    x_in = nc.dram_tensor("x", [b_per, c, t], DT, kind="ExternalInput")
    w_in = nc.dram_tensor("weights", [c], DT, kind="ExternalInput")
    y_out = nc.dram_tensor("out", [b_per, c, t], DT, kind="ExternalOutput")

    xr = x_in.ap().rearrange("b c t -> (b c) t")   # [rows, t]
    yr = y_out.ap().rearrange("b c t -> (b c) t")
    # w4[p, j] = weights[j*128 + p] — column j holds channel block j
    wr = w_in.ap().rearrange("(j p) -> p j", p=P)  # [128, c_blocks]

    with tile.TileContext(nc) as tc:
        with (
            tc.tile_pool(name="const", bufs=1) as cpool,
            tc.tile_pool(name="xp", bufs=6) as xpool,
            tc.tile_pool(name="xh", bufs=4) as hpool,
        ):
            w4 = cpool.tile([P, c_blocks], DT)
            s4 = cpool.tile([P, c_blocks], DT)
            a4 = cpool.tile([P, c_blocks], DT)
            dmy = cpool.tile([P, 1], DT)

            # Hoist the one-time ACT table load into the preamble window.
            nc.gpsimd.memset(dmy[:], 0.0)
            nc.scalar.activation(dmy[:], dmy[:], ACT_COPY, scale=1.0)

            # Weights DMA off the sync ring so x block 0 leads the wire.
            # (HWDGE issuers are SP and Activation; scalar's queue is free.)
            nc.scalar.dma_start(w4[:], wr)
            # s = clamp(w, 0, 1); a = 1 - s  (gpsimd: keeps Vector scan-only)
            nc.gpsimd.tensor_scalar(s4[:], w4[:], 0.0, 1.0, OP.max, OP.min)
            nc.gpsimd.tensor_scalar(a4[:], s4[:], -1.0, 1.0, OP.mult, OP.add)

            def premul_scan(xt, lo, hi, j, first, init):
                # Premultiply s*x in place; for the row start skip col 0 so
                # the scan's first step gives state_0 = a*0 + x_0 exactly.
                nc.scalar.activation(
                    xt[:, lo + (1 if first else 0):hi], xt[:, lo + (1 if first else 0):hi],
                    ACT_COPY, scale=s4[:, j:j + 1],
                )
                nc.vector.tensor_tensor_scan(
                    xt[:, lo:hi],
                    a4[:, j:j + 1].to_broadcast((P, hi - lo)),
                    xt[:, lo:hi],
                    init,
                    OP.mult,
                    OP.add,
                )

            for k in range(n_blocks):
                j = k % c_blocks  # channel block of rows [k*128, (k+1)*128)
                r0 = k * P
                if k in (0, n_blocks - 1):
                    # Half-T pieces in separate tiles: shorter pipeline fill
                    # (k=0) and drain (last block) on the critical path.
                    xa = hpool.tile([P, th], DT)
                    xb = hpool.tile([P, th], DT)
                    nc.sync.dma_start(xa[:], xr[r0:r0 + P, 0:th])
                    nc.sync.dma_start(xb[:], xr[r0:r0 + P, th:t])
                    premul_scan(xa, 0, th, j, True, 0.0)
                    nc.gpsimd.dma_start(yr[r0:r0 + P, 0:th], xa[:])
                    premul_scan(xb, 0, th, j, False, xa[:, th - 1:th])
                    nc.gpsimd.dma_start(yr[r0:r0 + P, th:t], xb[:])
                else:
                    xt = xpool.tile([P, t], DT)
                    nc.sync.dma_start(xt[:], xr[r0:r0 + P, :])
                    premul_scan(xt, 0, t, j, True, 0.0)
                    nc.gpsimd.dma_start(yr[r0:r0 + P, :], xt[:])
    nc.compile()
    return nc


_NC_CACHE = []


def kernel(x, weights, _run_kwargs=None):
    if not _NC_CACHE:
        _NC_CACHE.append(build())
    nc = _NC_CACHE[0]
    x = np.ascontiguousarray(np.asarray(x, dtype=np.float32))
    weights = np.ascontiguousarray(np.asarray(weights, dtype=np.float32))
    in_maps = [
        {"x": x[i * B_PER:(i + 1) * B_PER], "weights": weights}
        for i in range(N_CORES)
    ]
    res = run_bass_kernel_spmd(
        nc, in_maps, core_ids=list(range(N_CORES)), **(_run_kwargs or {})
    )
    out = np.concatenate([res.results[i]["out"] for i in range(N_CORES)], axis=0)
    if _run_kwargs:
        kernel.last_results = res
    return out
